# revision 8
# baseline (speedup 1.0000x reference)
"""Trainium2 Bass kernel for nn_CustomerizedLoss (MSE + per-sample weight-conditioned
MLP cross-entropy over a fixed image set).

Sharding: model-batch dim B=64 split across 8 NeuronCores (8 samples each);
the 10000x784 image matrix is replicated (shipped transposed, bf16).

Per core:
  mm1:  h^T[bh=512, n] = W1T[785, 512]^T @ imagesT_ext[785, n]   (bias via ones-row)
  relu: ScalarE psum->sbuf bf16
  mm2:  logits[n, 80] = h^T^T @ W2blk[512, 80] + ones-row @ B2   (block-diag W2)
  CE:   grouped (8 groups of 10) log-softmax + one-hot label dot, accumulated
  loss1: sum((inp1-tar1)^2) over this core's 8 rows
Host combines partial sums into (combined, loss1, loss2).
"""

import numpy as np
import ml_dtypes

BF16 = ml_dtypes.bfloat16
FP8 = ml_dtypes.float8_e4m3

INPUT, HIDDEN, OUT = 784, 64, 10
NTEST, B, WVEC = 10000, 64, 50890
NCORES = 8
BLOC = B // NCORES          # 8 samples per core
BH = BLOC * HIDDEN          # 512
NPAD = 10240                # images padded to 20*512
NCHUNK = 20
CW = 512                    # n-chunk width
KC = 7                      # contraction chunks (112 each; chunk 0 has +1 bias row)
L1N = BLOC * WVEC           # 407120
L1COLS = -(-L1N // 128)     # 3181

_CACHE = {}


def _build():
    from contextlib import ExitStack
    import concourse.bass as bass
    from concourse import bacc
    import concourse.mybir as mybir
    import concourse.tile as tile

    f32 = mybir.dt.float32
    bf = mybir.dt.bfloat16
    fp8 = mybir.dt.float8e4
    AX = mybir.AxisListType.X
    OP = mybir.AluOpType
    ACT = mybir.ActivationFunctionType

    nc = bacc.Bacc("TRN2", target_bir_lowering=False, num_devices=NCORES)

    imt_d = nc.declare_dram_parameter("imt", [NCHUNK, 128, KC, CW], fp8, isOutput=False)
    w1t_d = nc.declare_dram_parameter("w1t", [128, KC, BH], fp8, isOutput=False)
    w2b_d = nc.declare_dram_parameter("w2b", [128, 4, 80], bf, isOutput=False)
    b2_d = nc.declare_dram_parameter("b2", [128, 320], bf, isOutput=False)
    oh_d = nc.declare_dram_parameter("oh", [NCHUNK, 128, 4 * 8 * 10], bf, isOutput=False)
    mask_d = nc.declare_dram_parameter("mask", [128, 32], f32, isOutput=False)
    x1_d = nc.declare_dram_parameter("x1", [128, L1COLS], f32, isOutput=False)
    t1_d = nc.declare_dram_parameter("t1", [128, L1COLS], f32, isOutput=False)
    ce_d = nc.declare_dram_parameter("out_ce", [128, 32], f32, isOutput=True)
    sq_d = nc.declare_dram_parameter("out_sq", [128, 1], f32, isOutput=True)

    with tile.TileContext(nc) as tc:
        with ExitStack() as ctx:
            persist = ctx.enter_context(tc.tile_pool(name="persist", bufs=1))
            im_pool = ctx.enter_context(tc.tile_pool(name="im", bufs=4))
            oh_pool = ctx.enter_context(tc.tile_pool(name="oh", bufs=4))
            h_pool = ctx.enter_context(tc.tile_pool(name="h", bufs=2))
            s_pool = ctx.enter_context(tc.tile_pool(name="s", bufs=2))
            pa_pool = ctx.enter_context(tc.tile_pool(name="pa", bufs=4, space="PSUM"))
            pb_pool = ctx.enter_context(tc.tile_pool(name="pb", bufs=3, space="PSUM"))

            w1t = persist.tile([128, KC, BH], fp8)
            nc.sync.dma_start(out=w1t, in_=w1t_d[:, :, :])
            w2b = persist.tile([128, 4, 80], bf)
            b2 = persist.tile([128, 32, 10], bf)
            mask = persist.tile([128, 32], f32)
            acc = persist.tile([128, 32], f32)
            nc.vector.memset(acc, 0.0)

            # log-sum-exp inputs collected across chunks; single Ln at the end
            # avoids per-chunk ACT table-set thrash (Exp vs Ln sets).
            ssum_all = persist.tile([128, 32, NCHUNK], f32)

            for c in range(NCHUNK):
                imt = im_pool.tile([128, KC, CW], fp8)
                nc.sync.dma_start(out=imt, in_=imt_d[c, :, :, :])
                oht = oh_pool.tile([128, 32, 10], bf)
                nc.sync.dma_start(
                    out=oht.rearrange("p g o -> p (g o)"), in_=oh_d[c, :, :]
                )
                if c == 0:
                    nc.sync.dma_start(out=w2b, in_=w2b_d[:, :, :])
                    nc.sync.dma_start(out=b2.rearrange("p g o -> p (g o)"), in_=b2_d[:, :])
                    nc.sync.dma_start(out=mask, in_=mask_d[:, :])

                ht = h_pool.tile([128, 4, CW], bf)
                for bh in range(4):
                    pa = pa_pool.tile([128, CW], f32)
                    # fp8 DoubleRow: pair k-subtiles (zero-padded rows are inert)
                    for kp in range(3):
                        nc.tensor.matmul(
                            pa[:, :],
                            w1t[:, 2 * kp:2 * kp + 2, bh * 128:(bh + 1) * 128],
                            imt[:, 2 * kp:2 * kp + 2, :],
                            start=(kp == 0), stop=False,
                            perf_mode=mybir.MatmulPerfMode.DoubleRow,
                        )
                    nc.tensor.matmul(
                        pa[:, :],
                        w1t[0:112, 6, bh * 128:(bh + 1) * 128],
                        imt[0:112, 6, :],
                        start=False, stop=True,
                    )
                    nc.scalar.activation(out=ht[:, bh, :], in_=pa[:, :], func=ACT.Relu)

                pb = pb_pool.tile([128, 32, 10], f32)
                for ns in range(4):
                    outap = pb[:, ns * 8:(ns + 1) * 8, :].rearrange("p g o -> p (g o)")
                    for j in range(4):
                        nc.tensor.matmul(
                            outap,
                            ht[:, j, ns * 128:(ns + 1) * 128],
                            w2b[:, j, :],
                            start=(j == 0), stop=(j == 3),
                        )

                P2 = s_pool.tile([128, 32, 10], f32)
                nc.vector.tensor_tensor(P2, pb, b2, OP.add)
                mx = s_pool.tile([128, 32], f32)
                nc.vector.tensor_reduce(out=mx, in_=P2, axis=AX, op=OP.max)
                S = s_pool.tile([128, 32, 10], f32)
                nc.vector.tensor_tensor(
                    S, P2, mx[:, :, None].broadcast_to([128, 32, 10]), OP.subtract
                )
                E = s_pool.tile([128, 32, 10], f32)
                nc.scalar.activation(out=E, in_=S, func=ACT.Exp)
                nc.vector.tensor_reduce(out=ssum_all[:, :, c], in_=E, axis=AX, op=OP.add)
                prod = s_pool.tile([128, 32, 10], f32)
                nc.gpsimd.tensor_tensor(prod, S, oht, OP.mult)
                dotv = s_pool.tile([128, 32], f32)
                nc.vector.tensor_reduce(out=dotv, in_=prod, axis=AX, op=OP.add)
                nc.vector.tensor_add(acc, acc, dotv)

                if c == NCHUNK - 2:
                    lse_all = persist.tile([128, 32, NCHUNK], f32)
                    nc.scalar.activation(
                        out=lse_all[:, :, 0:NCHUNK - 1],
                        in_=ssum_all[:, :, 0:NCHUNK - 1], func=ACT.Ln,
                    )
                if c == 0:
                    x1 = persist.tile([128, L1COLS], f32)
                    nc.sync.dma_start(out=x1, in_=x1_d[:, :])
                    t1 = persist.tile([128, L1COLS], f32)
                    nc.sync.dma_start(out=t1, in_=t1_d[:, :])
                if c == 2:
                    sq = persist.tile([128, 1], f32)
                    nc.vector.tensor_sub(x1, x1, t1)
                    nc.scalar.activation(out=t1, in_=x1, func=ACT.Square)
                    nc.vector.tensor_reduce(out=sq, in_=t1, axis=AX, op=OP.add)
                    nc.sync.dma_start(out=sq_d[:, :], in_=sq)

            # tail: lse (bulk was done after chunk 18), mask pads, combine
            nc.scalar.activation(
                out=lse_all[:, :, NCHUNK - 1], in_=ssum_all[:, :, NCHUNK - 1],
                func=ACT.Ln,
            )
            nc.vector.tensor_mul(lse_all[:, :, NCHUNK - 1], lse_all[:, :, NCHUNK - 1], mask)
            lsum = persist.tile([128, 32], f32)
            nc.vector.tensor_reduce(out=lsum, in_=lse_all, axis=AX, op=OP.add)
            nc.vector.tensor_sub(lsum, lsum, acc)
            nc.sync.dma_start(out=ce_d[:, :], in_=lsum)

    nc.compile()
    return nc


def _prep_shared(images):
    """imt [NCHUNK, 113, KC, CW] bf16 (chunk-major so each chunk is one
    contiguous 810KB slab -> DMA sprays across all 16 engines):
    imagesT in 112-row chunks + ones/zeros bias row."""
    imt = np.zeros((128, KC, NPAD), dtype=np.float32)
    a = images.T.reshape(KC, 112, NTEST).transpose(1, 0, 2)  # [112, KC, NTEST]
    imt[:112, :, :NTEST] = a
    imt[112, 0, :] = 1.0
    imt = imt.reshape(128, KC, NCHUNK, CW).transpose(2, 0, 1, 3)
    return np.ascontiguousarray(imt.astype(FP8))


def _prep_core(inp1, tar1, inp2, tar2):
    """Per-core input dict from this core's 8-sample slices."""
    o1 = INPUT * HIDDEN
    o2 = o1 + HIDDEN
    o3 = o2 + HIDDEN * OUT
    W1 = inp2[:, :o1].reshape(BLOC, HIDDEN, INPUT)
    B1 = inp2[:, o1:o2].reshape(BH)
    W2 = inp2[:, o2:o3].reshape(BLOC, OUT, HIDDEN)
    B2 = inp2[:, o3:].reshape(1, BLOC * OUT)

    w1t = np.zeros((128, KC, BH), dtype=np.float32)
    # W1 [b,h,d] -> [d, b*64+h] -> chunks [112, KC, BH]
    w1t[:112] = W1.reshape(BH, KC, 112).transpose(2, 1, 0)
    w1t[112, 0, :] = B1

    w2blk = np.zeros((BH, BLOC * OUT), dtype=np.float32)
    for b in range(BLOC):
        w2blk[b * HIDDEN:(b + 1) * HIDDEN, b * OUT:(b + 1) * OUT] = W2[b].T
    w2b = w2blk.reshape(4, 128, 80).transpose(1, 0, 2)

    # one-hot labels in device layout [NCHUNK, 128, 4*8*10]
    oh = np.zeros((BLOC, NPAD, OUT), dtype=np.float32)
    oh[np.arange(BLOC)[:, None], np.arange(NTEST)[None, :], tar2.astype(np.int64)] = 1.0
    # [b, chunk, ns, p, o] -> [chunk, p, ns, b, o]
    ohd = oh.reshape(BLOC, NCHUNK, 4, 128, OUT).transpose(1, 3, 2, 0, 4)
    ohd = ohd.reshape(NCHUNK, 128, 4 * BLOC * OUT)

    mask = np.zeros((128, 32), dtype=np.float32)
    n0 = (NCHUNK - 1) * CW
    for ns in range(4):
        valid = np.clip(NTEST - (n0 + ns * 128), 0, 128)
        mask[:valid, ns * 8:(ns + 1) * 8] = 1.0

    x1 = np.zeros((128 * L1COLS,), dtype=np.float32)
    x1[:L1N] = inp1.ravel()
    t1 = np.zeros((128 * L1COLS,), dtype=np.float32)
    t1[:L1N] = tar1.ravel()

    return {
        "w1t": np.ascontiguousarray(w1t.astype(FP8)),
        "w2b": np.ascontiguousarray(w2b.astype(BF16)),
        "b2": np.ascontiguousarray(np.tile(B2.reshape(-1), (128, 4)).astype(BF16)),
        "oh": np.ascontiguousarray(ohd.astype(BF16)),
        "mask": mask,
        "x1": x1.reshape(128, L1COLS),
        "t1": t1.reshape(128, L1COLS),
    }


def kernel(inp1, tar1, inp2, tar2, images, _want_results=False):
    from concourse.bass_utils import run_bass_kernel_spmd

    inp1 = np.asarray(inp1, dtype=np.float32)
    tar1 = np.asarray(tar1, dtype=np.float32)
    inp2 = np.asarray(inp2, dtype=np.float32)
    tar2 = np.asarray(tar2)
    images = np.asarray(images, dtype=np.float32)

    if "nc" not in _CACHE:
        _CACHE["nc"] = _build()
    nc = _CACHE["nc"]

    imt = _prep_shared(images)
    in_maps = []
    for core in range(NCORES):
        s = slice(core * BLOC, (core + 1) * BLOC)
        m = _prep_core(inp1[s], tar1[s], inp2[s], tar2[s])
        m["imt"] = imt
        in_maps.append(m)

    res = run_bass_kernel_spmd(nc, in_maps, core_ids=list(range(NCORES)))

    ce_sum = 0.0
    sq_sum = 0.0
    for core in range(NCORES):
        ce_sum += np.sum(res.results[core]["out_ce"].astype(np.float64))
        sq_sum += np.sum(res.results[core]["out_sq"].astype(np.float64))

    loss1 = 20.0 * sq_sum / (B * WVEC)
    loss2 = ce_sum / (B * NTEST)
    combined = loss1 + loss2
    out = (
        np.float32(combined),
        np.float32(loss1),
        np.float32(loss2),
    )
    if _want_results:
        return out, res
    return out


# revision 9
# speedup vs baseline: 1.0247x; 1.0247x over previous
"""Trainium2 Bass kernel for nn_CustomerizedLoss (MSE + per-sample weight-conditioned
MLP cross-entropy over a fixed image set).

Sharding: model-batch dim B=64 split across 8 NeuronCores (8 samples each);
the 10000x784 image matrix is replicated (shipped transposed, bf16).

Per core:
  mm1:  h^T[bh=512, n] = W1T[785, 512]^T @ imagesT_ext[785, n]   (bias via ones-row)
  relu: ScalarE psum->sbuf bf16
  mm2:  logits[n, 80] = h^T^T @ W2blk[512, 80] + ones-row @ B2   (block-diag W2)
  CE:   grouped (8 groups of 10) log-softmax + one-hot label dot, accumulated
  loss1: sum((inp1-tar1)^2) over this core's 8 rows
Host combines partial sums into (combined, loss1, loss2).
"""

import numpy as np
import ml_dtypes

BF16 = ml_dtypes.bfloat16
FP8 = ml_dtypes.float8_e4m3

INPUT, HIDDEN, OUT = 784, 64, 10
NTEST, B, WVEC = 10000, 64, 50890
NCORES = 8
BLOC = B // NCORES          # 8 samples per core
BH = BLOC * HIDDEN          # 512
NPAD = 10240                # images padded to 20*512
NCHUNK = 20
CW = 512                    # n-chunk width
KC = 7                      # contraction chunks (112 each; chunk 0 has +1 bias row)
L1N = BLOC * WVEC           # 407120
L1COLS = -(-L1N // 128)     # 3181

_CACHE = {}


def _build():
    from contextlib import ExitStack
    import concourse.bass as bass
    from concourse import bacc
    import concourse.mybir as mybir
    import concourse.tile as tile

    f32 = mybir.dt.float32
    bf = mybir.dt.bfloat16
    fp8 = mybir.dt.float8e4
    AX = mybir.AxisListType.X
    OP = mybir.AluOpType
    ACT = mybir.ActivationFunctionType

    nc = bacc.Bacc("TRN2", target_bir_lowering=False, num_devices=NCORES)

    imt_d = nc.declare_dram_parameter("imt", [NCHUNK, 128, KC, CW], fp8, isOutput=False)
    w1t_d = nc.declare_dram_parameter("w1t", [128, KC, BH], fp8, isOutput=False)
    w2b_d = nc.declare_dram_parameter("w2b", [128, 4, 80], bf, isOutput=False)
    b2_d = nc.declare_dram_parameter("b2", [128, 320], bf, isOutput=False)
    oh_d = nc.declare_dram_parameter("oh", [NCHUNK, 128, 4 * 8 * 10], bf, isOutput=False)
    mask_d = nc.declare_dram_parameter("mask", [128, 32], f32, isOutput=False)
    x1_d = nc.declare_dram_parameter("x1", [128, L1COLS], f32, isOutput=False)
    t1_d = nc.declare_dram_parameter("t1", [128, L1COLS], f32, isOutput=False)
    ce_d = nc.declare_dram_parameter("out_ce", [128, 32], f32, isOutput=True)
    sq_d = nc.declare_dram_parameter("out_sq", [128, 1], f32, isOutput=True)

    with tile.TileContext(nc) as tc:
        with ExitStack() as ctx:
            persist = ctx.enter_context(tc.tile_pool(name="persist", bufs=1))
            im_pool = ctx.enter_context(tc.tile_pool(name="im", bufs=4))
            oh_pool = ctx.enter_context(tc.tile_pool(name="oh", bufs=4))
            h_pool = ctx.enter_context(tc.tile_pool(name="h", bufs=2))
            s_pool = ctx.enter_context(tc.tile_pool(name="s", bufs=2))
            pa_pool = ctx.enter_context(tc.tile_pool(name="pa", bufs=4, space="PSUM"))
            pb_pool = ctx.enter_context(tc.tile_pool(name="pb", bufs=3, space="PSUM"))

            w1t = persist.tile([128, KC, BH], fp8)
            nc.sync.dma_start(out=w1t, in_=w1t_d[:, :, :])
            w2b = persist.tile([128, 4, 80], bf)
            b2 = persist.tile([128, 32, 10], bf)
            mask = persist.tile([128, 32], f32)
            acc = persist.tile([128, 32], f32)
            nc.vector.memset(acc, 0.0)

            # log-sum-exp inputs collected across chunks; single Ln at the end
            # avoids per-chunk ACT table-set thrash (Exp vs Ln sets).
            ssum_all = persist.tile([128, 32, NCHUNK], f32)

            for c in range(NCHUNK):
                imt = im_pool.tile([128, KC, CW], fp8)
                nc.sync.dma_start(out=imt, in_=imt_d[c, :, :, :])
                oht = oh_pool.tile([128, 32, 10], bf)
                nc.sync.dma_start(
                    out=oht.rearrange("p g o -> p (g o)"), in_=oh_d[c, :, :]
                )
                if c == 0:
                    nc.sync.dma_start(out=w2b, in_=w2b_d[:, :, :])
                    nc.sync.dma_start(out=b2.rearrange("p g o -> p (g o)"), in_=b2_d[:, :])
                    nc.sync.dma_start(out=mask, in_=mask_d[:, :])

                ht = h_pool.tile([128, 4, CW], bf)
                for bh in range(4):
                    pa = pa_pool.tile([128, CW], f32)
                    # fp8 DoubleRow: pair k-subtiles (zero-padded rows are inert)
                    for kp in range(3):
                        nc.tensor.matmul(
                            pa[:, :],
                            w1t[:, 2 * kp:2 * kp + 2, bh * 128:(bh + 1) * 128],
                            imt[:, 2 * kp:2 * kp + 2, :],
                            start=(kp == 0), stop=False,
                            perf_mode=mybir.MatmulPerfMode.DoubleRow,
                        )
                    nc.tensor.matmul(
                        pa[:, :],
                        w1t[0:112, 6, bh * 128:(bh + 1) * 128],
                        imt[0:112, 6, :],
                        start=False, stop=True,
                    )
                    nc.scalar.activation(out=ht[:, bh, :], in_=pa[:, :], func=ACT.Relu)

                pb = pb_pool.tile([128, 32, 10], f32)
                for ns in range(4):
                    outap = pb[:, ns * 8:(ns + 1) * 8, :].rearrange("p g o -> p (g o)")
                    for j in range(4):
                        nc.tensor.matmul(
                            outap,
                            ht[:, j, ns * 128:(ns + 1) * 128],
                            w2b[:, j, :],
                            start=(j == 0), stop=(j == 3),
                        )

                P2 = s_pool.tile([128, 32, 10], f32)
                nc.vector.tensor_tensor(P2, pb, b2, OP.add)
                mx = s_pool.tile([128, 32], f32)
                nc.vector.tensor_reduce(out=mx, in_=P2, axis=AX, op=OP.max)
                S = s_pool.tile([128, 32, 10], f32)
                nc.vector.tensor_tensor(
                    S, P2, mx[:, :, None].broadcast_to([128, 32, 10]), OP.subtract
                )
                E = s_pool.tile([128, 32, 10], f32)
                nc.scalar.activation(out=E, in_=S, func=ACT.Exp)
                nc.vector.tensor_reduce(out=ssum_all[:, :, c], in_=E, axis=AX, op=OP.add)
                prod = s_pool.tile([128, 32, 10], f32)
                nc.vector.tensor_tensor(prod, S, oht, OP.mult)
                dotv = s_pool.tile([128, 32], f32)
                nc.vector.tensor_reduce(out=dotv, in_=prod, axis=AX, op=OP.add)
                nc.vector.tensor_add(acc, acc, dotv)

                if c == NCHUNK - 2:
                    lse_all = persist.tile([128, 32, NCHUNK], f32)
                    nc.scalar.activation(
                        out=lse_all[:, :, 0:NCHUNK - 1],
                        in_=ssum_all[:, :, 0:NCHUNK - 1], func=ACT.Ln,
                    )
                if c == 0:
                    x1 = persist.tile([128, L1COLS], f32)
                    nc.sync.dma_start(out=x1, in_=x1_d[:, :])
                    t1 = persist.tile([128, L1COLS], f32)
                    nc.sync.dma_start(out=t1, in_=t1_d[:, :])
                if c == 2:
                    sq = persist.tile([128, 1], f32)
                    nc.vector.tensor_sub(x1, x1, t1)
                    nc.scalar.activation(out=t1, in_=x1, func=ACT.Square)
                    nc.vector.tensor_reduce(out=sq, in_=t1, axis=AX, op=OP.add)
                    nc.sync.dma_start(out=sq_d[:, :], in_=sq)

            # tail: lse (bulk was done after chunk 18), mask pads, combine
            nc.scalar.activation(
                out=lse_all[:, :, NCHUNK - 1], in_=ssum_all[:, :, NCHUNK - 1],
                func=ACT.Ln,
            )
            nc.vector.tensor_mul(lse_all[:, :, NCHUNK - 1], lse_all[:, :, NCHUNK - 1], mask)
            lsum = persist.tile([128, 32], f32)
            nc.vector.tensor_reduce(out=lsum, in_=lse_all, axis=AX, op=OP.add)
            nc.vector.tensor_sub(lsum, lsum, acc)
            nc.sync.dma_start(out=ce_d[:, :], in_=lsum)

    nc.compile()
    return nc


def _prep_shared(images):
    """imt [NCHUNK, 113, KC, CW] bf16 (chunk-major so each chunk is one
    contiguous 810KB slab -> DMA sprays across all 16 engines):
    imagesT in 112-row chunks + ones/zeros bias row."""
    imt = np.zeros((128, KC, NPAD), dtype=np.float32)
    a = images.T.reshape(KC, 112, NTEST).transpose(1, 0, 2)  # [112, KC, NTEST]
    imt[:112, :, :NTEST] = a
    imt[112, 0, :] = 1.0
    imt = imt.reshape(128, KC, NCHUNK, CW).transpose(2, 0, 1, 3)
    return np.ascontiguousarray(imt.astype(FP8))


def _prep_core(inp1, tar1, inp2, tar2):
    """Per-core input dict from this core's 8-sample slices."""
    o1 = INPUT * HIDDEN
    o2 = o1 + HIDDEN
    o3 = o2 + HIDDEN * OUT
    W1 = inp2[:, :o1].reshape(BLOC, HIDDEN, INPUT)
    B1 = inp2[:, o1:o2].reshape(BH)
    W2 = inp2[:, o2:o3].reshape(BLOC, OUT, HIDDEN)
    B2 = inp2[:, o3:].reshape(1, BLOC * OUT)

    w1t = np.zeros((128, KC, BH), dtype=np.float32)
    # W1 [b,h,d] -> [d, b*64+h] -> chunks [112, KC, BH]
    w1t[:112] = W1.reshape(BH, KC, 112).transpose(2, 1, 0)
    w1t[112, 0, :] = B1

    w2blk = np.zeros((BH, BLOC * OUT), dtype=np.float32)
    for b in range(BLOC):
        w2blk[b * HIDDEN:(b + 1) * HIDDEN, b * OUT:(b + 1) * OUT] = W2[b].T
    w2b = w2blk.reshape(4, 128, 80).transpose(1, 0, 2)

    # one-hot labels in device layout [NCHUNK, 128, 4*8*10]
    oh = np.zeros((BLOC, NPAD, OUT), dtype=np.float32)
    oh[np.arange(BLOC)[:, None], np.arange(NTEST)[None, :], tar2.astype(np.int64)] = 1.0
    # [b, chunk, ns, p, o] -> [chunk, p, ns, b, o]
    ohd = oh.reshape(BLOC, NCHUNK, 4, 128, OUT).transpose(1, 3, 2, 0, 4)
    ohd = ohd.reshape(NCHUNK, 128, 4 * BLOC * OUT)

    mask = np.zeros((128, 32), dtype=np.float32)
    n0 = (NCHUNK - 1) * CW
    for ns in range(4):
        valid = np.clip(NTEST - (n0 + ns * 128), 0, 128)
        mask[:valid, ns * 8:(ns + 1) * 8] = 1.0

    x1 = np.zeros((128 * L1COLS,), dtype=np.float32)
    x1[:L1N] = inp1.ravel()
    t1 = np.zeros((128 * L1COLS,), dtype=np.float32)
    t1[:L1N] = tar1.ravel()

    return {
        "w1t": np.ascontiguousarray(w1t.astype(FP8)),
        "w2b": np.ascontiguousarray(w2b.astype(BF16)),
        "b2": np.ascontiguousarray(np.tile(B2.reshape(-1), (128, 4)).astype(BF16)),
        "oh": np.ascontiguousarray(ohd.astype(BF16)),
        "mask": mask,
        "x1": x1.reshape(128, L1COLS),
        "t1": t1.reshape(128, L1COLS),
    }


def kernel(inp1, tar1, inp2, tar2, images, _want_results=False):
    from concourse.bass_utils import run_bass_kernel_spmd

    inp1 = np.asarray(inp1, dtype=np.float32)
    tar1 = np.asarray(tar1, dtype=np.float32)
    inp2 = np.asarray(inp2, dtype=np.float32)
    tar2 = np.asarray(tar2)
    images = np.asarray(images, dtype=np.float32)

    if "nc" not in _CACHE:
        _CACHE["nc"] = _build()
    nc = _CACHE["nc"]

    imt = _prep_shared(images)
    in_maps = []
    for core in range(NCORES):
        s = slice(core * BLOC, (core + 1) * BLOC)
        m = _prep_core(inp1[s], tar1[s], inp2[s], tar2[s])
        m["imt"] = imt
        in_maps.append(m)

    res = run_bass_kernel_spmd(nc, in_maps, core_ids=list(range(NCORES)))

    ce_sum = 0.0
    sq_sum = 0.0
    for core in range(NCORES):
        ce_sum += np.sum(res.results[core]["out_ce"].astype(np.float64))
        sq_sum += np.sum(res.results[core]["out_sq"].astype(np.float64))

    loss1 = 20.0 * sq_sum / (B * WVEC)
    loss2 = ce_sum / (B * NTEST)
    combined = loss1 + loss2
    out = (
        np.float32(combined),
        np.float32(loss1),
        np.float32(loss2),
    )
    if _want_results:
        return out, res
    return out
